# revision 8
# baseline (speedup 1.0000x reference)
"""Trainium2 Bass kernel for nn_ControlGate (bank-selected virtual linear
projection + sigmoid gate), distributed over 8 NeuronCores.

Math (per batch element b):
    W_eff = sum_k sel_probs[b,k] * W[sel_idx[b,k]]      # (d_model, d_out)
    b_eff = sum_k sel_probs[b,k] * b[sel_idx[b,k]]      # (d_out,)
    out[b] = sigmoid(tensor[b] @ W_eff + b_eff)          # (seq, d_out)

Sharding: batch==8 maps 1:1 onto the 8 cores (data parallel). The bank
gather + probability-weighted superposition is tiny (2 x 4 MB per batch)
and runs on the host during input sharding; each core receives its batch's
token slab pre-transposed to contraction-major in bf16 plus the 2 MB
superposed W_eff (bf16) and fp32 bias. The device does the 4096x1024x1024
matmul (bf16 operands, fp32 PSUM accumulation), bias add and sigmoid.

bf16 operands double the PE moving-column rate vs fp32r (measured: the
full 4096x1024x1024 matmul sustains ~55 us vs the ~109 us fp32r floor) and
halve the token-stream + weight DMA traffic. The output is computed in
fp32 PSUM but stored to DRAM in bf16 (host widens back to fp32; adds
~2e-3 abs err vs the 2e-2 gate), keeping steady-state DMA (~18 MB/core)
under the PE roofline.
"""

import os
import sys

import numpy as np
import ml_dtypes

for _p in ("/opt/trn_rl_repo", "/root/.axon_site/_ro/trn_rl_repo"):
    if _p not in sys.path and os.path.isdir(_p):
        sys.path.insert(0, _p)

import concourse.bass as bass  # noqa: E402
import concourse.tile as tile  # noqa: E402
from concourse import bacc, mybir  # noqa: E402
from concourse.bass_utils import run_bass_kernel_spmd  # noqa: E402

# Problem shape (hardcoded per contract)
B, S, D = 8, 4096, 1024          # batch, seq, d_model
O = 1024                         # d_out = num_heads * prod(out_shape)
NUM_HEADS, D_HEAD = 16, 64
TOP_K = 2
N_CORES = 8

P = 128                          # SBUF partitions
KT = D // P                      # 8 contraction tiles
S_SUPER = 512                    # seq columns fetched per DMA super-chunk
N_SUPER = S // S_SUPER
S_SUB = S_SUPER // P             # 4 lhsT slices per super-chunk
ON = 512                         # output columns per PSUM bank
OH = O // ON                     # 2 output halves

F32 = mybir.dt.float32
BF16 = mybir.dt.bfloat16
BF16_NP = ml_dtypes.bfloat16

_PROGRAM = None


def _build_program(bench_reps=None, mode="full"):
    """Build + compile the single-core Bass program (same NEFF on all 8 cores).

    bench_reps: when set, builds a timing-only variant — the big inputs and
    the output live in Internal DRAM (no host transfer) and the whole body
    repeats bench_reps times in a device-side loop. mode: "full" | "dma"
    (DMAs only) | "pe" (matmuls only) — roofline calibration variants.
    """
    bench = bench_reps is not None
    DO_PE = mode in ("full", "pe")
    DO_DMA = mode in ("full", "dma")
    big = {} if not bench else {"kind": "Internal"}
    nc = bacc.Bacc(
        "TRN2", target_bir_lowering=False, debug=False, num_devices=N_CORES
    )
    xT = nc.dram_tensor("xT", [D, S], BF16, **({"kind": "ExternalInput"} if not bench else big))
    wf = nc.dram_tensor("wf", [D, O], BF16, **({"kind": "ExternalInput"} if not bench else big))
    pb = nc.dram_tensor("pb", [P, TOP_K], F32, kind="ExternalInput")
    bf = nc.dram_tensor("bf", [1, O], F32, **({"kind": "ExternalInput"} if not bench else big))
    out = nc.dram_tensor("out", [S, O], BF16, **({"kind": "ExternalOutput"} if not bench else big))
    tok = nc.dram_tensor("tok", [1, TOP_K], F32, kind="ExternalOutput") if bench else None

    with tile.TileContext(nc) as tc:
        from contextlib import ExitStack

        with ExitStack() as ctx:
            consts = ctx.enter_context(tc.tile_pool(name="consts", bufs=2))
            weffp = ctx.enter_context(tc.tile_pool(name="weff", bufs=2))
            xpool = ctx.enter_context(tc.tile_pool(name="x", bufs=3))
            opool = ctx.enter_context(tc.tile_pool(name="o", bufs=2))
            pspool = ctx.enter_context(
                tc.tile_pool(name="ps", bufs=1, space="PSUM")
            )

            if bench:
                ctx.enter_context(tc.For_i(0, bench_reps, 1))

            # Two HWDGE rings: x streaming on the SP ring; weights, bias and
            # output stores on the ACT ring, so the 8 MB token stream never
            # queues behind the 2 MB weight prefix (and vice versa).
            xT_r = xT.ap().rearrange("(c p) s -> p c s", p=P)

            # First token super-chunk goes out on the SP ring immediately.
            xs0 = xpool.tile([P, KT, S_SUPER], BF16, tag="xs")
            if DO_DMA:
                nc.sync.dma_start(xs0[:], xT_r[:, :, 0:S_SUPER])

            # Host-superposed W_eff streams in k-tile chunks on the ACT ring:
            # a small leading chunk so the PE can start as soon as xs0 lands,
            # then the bulk.
            weff = []
            w_dmas = []
            W_CHUNKS = globals().get("_W_CHUNKS", [(0, 1), (1, 3), (4, 4)])
            wf_r = wf.ap().rearrange("(c p) o -> p c o", p=P)  # (128, 8, O)
            for h, (k0, kn) in enumerate(W_CHUNKS):
                wk = weffp.tile([P, kn, O], BF16, tag=f"wc{h}", name=f"wc{h}")
                if DO_DMA:
                    w_dmas.append(nc.scalar.dma_start(wk[:], wf_r[:, k0 : k0 + kn, :]))
                for j in range(kn):
                    weff.append(wk[:, j, :])

            # Effective bias, replicated on every partition: the DMA reads the
            # (1, O) bias row once per partition via a 0-stride AP. Rides the
            # ACT ring behind the weight chunks (not needed until first drain).
            if mode == "full":
                bb_t = consts.tile([P, 1, O], F32)
                nc.scalar.dma_start(bb_t[:], bf.ap().partition_broadcast(P))
                bias_t = bb_t[:, 0, :]

            # Main loop: stream token columns, matmul against the resident
            # W_eff in bf16 (full-rate path, fp32 PSUM), bias + sigmoid, store.
            #
            # ss=0 runs its 8 PSUM accumulation groups k-outer (wave per
            # contraction tile) so the PE consumes each weff[k] the moment it
            # lands instead of serializing whole groups behind weff[7].
            out_r = out.ap().rearrange("(c p) o -> p c o", p=P)
            groups = [(sub, oh) for sub in range(S_SUB) for oh in range(OH)]
            for ss in range(N_SUPER):
                if ss == 0:
                    xs = xs0
                else:
                    cols = slice(ss * S_SUPER, (ss + 1) * S_SUPER)
                    xs = xpool.tile([P, KT, S_SUPER], BF16, tag="xs")
                    if DO_DMA:
                        nc.sync.dma_start(xs[:], xT_r[:, :, cols])
                ostage = opool.tile([P, S_SUB, O], BF16)

                def drain(ps, sub, oh):
                    if mode != "full":
                        return
                    osl = slice(oh * ON, (oh + 1) * ON)
                    nc.vector.tensor_add(ps[:], ps[:], bias_t[:, osl])
                    nc.scalar.activation(
                        ostage[:, sub, osl], ps[:],
                        mybir.ActivationFunctionType.Sigmoid,
                    )

                def store():
                    if not DO_DMA:
                        return
                    if ss == N_SUPER - 1:
                        for sub in range(S_SUB):
                            nc.scalar.dma_start(
                                out_r[:, ss * S_SUB + sub, :], ostage[:, sub, :]
                            )
                    else:
                        nc.scalar.dma_start(
                            out_r[:, ss * S_SUB : (ss + 1) * S_SUB, :], ostage[:]
                        )

                if not DO_PE:
                    for g, (sub, oh) in enumerate(groups):
                        drain(None, sub, oh)
                    store()
                elif ss == 0:
                    pss = [pspool.tile([P, ON], F32, name=f"ps{g}", tag=f"ps{g}") for g in range(len(groups))]
                    for k in range(KT):
                        for g, (sub, oh) in enumerate(groups):
                            nc.tensor.matmul(
                                pss[g],
                                xs[:, k, sub * P : (sub + 1) * P],
                                weff[k][:, oh * ON : (oh + 1) * ON],
                                start=(k == 0),
                                stop=(k == KT - 1),
                            )
                    for g, (sub, oh) in enumerate(groups):
                        drain(pss[g], sub, oh)
                    store()
                else:
                    for g, (sub, oh) in enumerate(groups):
                        ps = pspool.tile([P, ON], F32, name=f"ps{g}", tag=f"ps{g}")
                        for k in range(KT):
                            nc.tensor.matmul(
                                ps[:],
                                xs[:, k, sub * P : (sub + 1) * P],
                                weff[k][:, oh * ON : (oh + 1) * ON],
                                start=(k == 0),
                                stop=(k == KT - 1),
                            )
                        drain(ps, sub, oh)
                    store()

        if tok is not None:
            nc.sync.dma_start(tok.ap(), pb.ap()[0:1, :])

    nc.compile()
    return nc


def _get_program():
    global _PROGRAM
    if _PROGRAM is None:
        _PROGRAM = _build_program()
    return _PROGRAM


def _make_in_maps(tensor, sel_idx, sel_probs, W, b):
    tensor = np.asarray(tensor, dtype=np.float32)
    sel_idx = np.asarray(sel_idx).astype(np.int64)
    sel_probs = np.asarray(sel_probs, dtype=np.float32)
    W = np.asarray(W, dtype=np.float32)
    b = np.asarray(b, dtype=np.float32)

    in_maps = []
    for c in range(N_CORES):
        idx = sel_idx[c]
        p = sel_probs[c]
        # Bank gather + superposition on host (2 x 4 MB per batch, trivial):
        # the device sees only the 2 MB effective weight matrix.
        weff = p[0] * W[idx[0]] + p[1] * W[idx[1]]          # (D, O) fp32
        beff = p[0] * b[idx[0]] + p[1] * b[idx[1]]          # (O,)   fp32
        in_maps.append(
            {
                "xT": np.ascontiguousarray(tensor[c].T).astype(BF16_NP),
                "wf": weff.astype(BF16_NP),
                "pb": np.ascontiguousarray(
                    np.broadcast_to(p[None, :], (P, TOP_K))
                ),
                "bf": beff[None, :],
            }
        )
    return in_maps


def _execute(in_maps, trace=False, **kwargs):
    nc = _get_program()
    return run_bass_kernel_spmd(
        nc, in_maps, core_ids=list(range(N_CORES)), trace=trace, **kwargs
    )


def kernel(tensor, sel_idx, sel_probs, W, b):
    in_maps = _make_in_maps(tensor, sel_idx, sel_probs, W, b)
    res = _execute(in_maps)
    out = np.stack(
        [res.results[c]["out"].astype(np.float32) for c in range(N_CORES)], axis=0
    )
    return out.reshape(B, S, NUM_HEADS, D_HEAD)


# revision 15
# speedup vs baseline: 1.6051x; 1.6051x over previous
"""Trainium2 Bass kernel for nn_ControlGate (bank-selected virtual linear
projection + sigmoid gate), distributed over 8 NeuronCores.

Math (per batch element b):
    W_eff = sum_k sel_probs[b,k] * W[sel_idx[b,k]]      # (d_model, d_out)
    b_eff = sum_k sel_probs[b,k] * b[sel_idx[b,k]]      # (d_out,)
    out[b] = sigmoid(tensor[b] @ W_eff + b_eff)          # (seq, d_out)

Sharding: batch==8 maps 1:1 onto the 8 cores (data parallel). The bank
gather + probability-weighted superposition is tiny (2 x 4 MB per batch)
and runs on the host during input sharding; each core receives its batch's
token slab pre-transposed to contraction-major in bf16 plus the 2 MB
superposed W_eff (bf16) and fp32 bias. The device does the 4096x1024x1024
matmul (bf16 operands, fp32 PSUM accumulation), bias add and sigmoid.

bf16 operands double the PE moving-column rate vs fp32r (measured: the
full 4096x1024x1024 matmul sustains ~55 us vs the ~109 us fp32r floor) and
halve the token-stream + weight DMA traffic. The output is computed in
fp32 PSUM but stored to DRAM in bf16 (host widens back to fp32; adds
~2e-3 abs err vs the 2e-2 gate), keeping steady-state DMA (~18 MB/core)
under the PE roofline.
"""

import os
import sys

import numpy as np
import ml_dtypes

for _p in ("/opt/trn_rl_repo", "/root/.axon_site/_ro/trn_rl_repo"):
    if _p not in sys.path and os.path.isdir(_p):
        sys.path.insert(0, _p)

import concourse.bass as bass  # noqa: E402
import concourse.tile as tile  # noqa: E402
from concourse import bacc, mybir  # noqa: E402
from concourse.bass_utils import run_bass_kernel_spmd  # noqa: E402

# Problem shape (hardcoded per contract)
B, S, D = 8, 4096, 1024          # batch, seq, d_model
O = 1024                         # d_out = num_heads * prod(out_shape)
NUM_HEADS, D_HEAD = 16, 64
TOP_K = 2
N_CORES = 8

P = 128                          # SBUF partitions
KT = D // P                      # 8 contraction tiles
S_SUPER = 512                    # seq columns fetched per DMA super-chunk
N_SUPER = S // S_SUPER
S_SUB = S_SUPER // P             # 4 lhsT slices per super-chunk
ON = 512                         # output columns per PSUM bank
OH = O // ON                     # 2 output halves

F32 = mybir.dt.float32
BF16 = mybir.dt.bfloat16
BF16_NP = ml_dtypes.bfloat16

# Experiment switches (defaults are the shipping configuration).
OUT_BF16 = os.environ.get("K_OUT_BF16", "1") == "1"
W_BUFS = int(os.environ.get("K_W_BUFS", "1"))
X_YIELDS_TO_W = os.environ.get("K_XDEP", "1") == "1"

_PROGRAM = None


def _build_program(bench_reps=None, mode="full"):
    """Build + compile the single-core Bass program (same NEFF on all 8 cores).

    bench_reps: when set, builds a timing-only variant — the big inputs and
    the output live in Internal DRAM (no host transfer) and the whole body
    repeats bench_reps times in a device-side loop. mode: "full" | "dma"
    (DMAs only) | "pe" (matmuls only) — roofline calibration variants.
    """
    bench = bench_reps is not None
    DO_PE = mode in ("full", "pe")
    DO_DMA = mode in ("full", "dma")
    big = {} if not bench else {"kind": "Internal"}
    nc = bacc.Bacc(
        "TRN2", target_bir_lowering=False, debug=False, num_devices=N_CORES
    )
    xT = nc.dram_tensor("xT", [D, S], BF16, **({"kind": "ExternalInput"} if not bench else big))
    wf = nc.dram_tensor("wf", [D, O], BF16, **({"kind": "ExternalInput"} if not bench else big))
    pb = nc.dram_tensor("pb", [P, TOP_K], F32, kind="ExternalInput")
    bf = nc.dram_tensor("bf", [1, O], F32, **({"kind": "ExternalInput"} if not bench else big))
    OUT_DT = BF16 if OUT_BF16 else F32
    out = nc.dram_tensor("out", [S, O], OUT_DT, **({"kind": "ExternalOutput"} if not bench else big))
    tok = nc.dram_tensor("tok", [1, TOP_K], F32, kind="ExternalOutput") if bench else None

    with tile.TileContext(nc) as tc:
        from contextlib import ExitStack

        with ExitStack() as ctx:
            consts = ctx.enter_context(tc.tile_pool(name="consts", bufs=1))
            weffp = ctx.enter_context(tc.tile_pool(name="weff", bufs=W_BUFS))
            xpool = ctx.enter_context(tc.tile_pool(name="x", bufs=3))
            opool = ctx.enter_context(tc.tile_pool(name="o", bufs=2))
            pspool = ctx.enter_context(
                tc.tile_pool(name="ps", bufs=1, space="PSUM")
            )

            xT_r = xT.ap().rearrange("(c p) s -> p c s", p=P)
            wf_r = wf.ap().rearrange("(c p) o -> p c o", p=P)  # (128, 8, O)
            W_CHUNKS = globals().get("_W_CHUNKS", [(0, 1), (1, 3), (4, 4)])

            # Calibration modes stage their never-recomputed operands ONCE,
            # outside the timed loop (a tile that is read but never written
            # fails allocation; filling it per-rep would pollute the slope).
            cal = bench and mode != "full"
            xs_cal = ost_cal = None
            weff = []
            w_dmas = []
            if cal:
                if mode == "pe":
                    xs_cal = xpool.tile([P, KT, S_SUPER], BF16, tag="xs")
                    nc.sync.dma_start(xs_cal[:], xT_r[:, :, 0:S_SUPER])
                    for h, (k0, kn) in enumerate(W_CHUNKS):
                        wk = weffp.tile([P, kn, O], BF16, tag=f"wc{h}", name=f"wc{h}")
                        nc.scalar.dma_start(wk[:], wf_r[:, k0 : k0 + kn, :])
                        for j in range(kn):
                            weff.append(wk[:, j, :])
                else:  # dma
                    assert OUT_DT == BF16, "dma calibration assumes bf16 ostage"
                    ost_cal = opool.tile([P, S_SUB, O], OUT_DT)
                    nc.scalar.dma_start(ost_cal[:], wf_r[:, 0:S_SUB, :])

            if bench:
                ctx.enter_context(tc.For_i(0, bench_reps, 1))

            # Two HWDGE rings: x streaming on the SP ring; weights, bias and
            # output stores on the ACT ring, so the 8 MB token stream never
            # queues behind the 2 MB weight prefix (and vice versa).

            # First token super-chunk goes out on the SP ring immediately.
            if mode == "pe":
                xs0 = xs_cal
            else:
                xs0 = xpool.tile([P, KT, S_SUPER], BF16, tag="xs")
                nc.sync.dma_start(xs0[:], xT_r[:, :, 0:S_SUPER])

            # Host-superposed W_eff streams in k-tile chunks on the ACT ring:
            # a small leading chunk so the PE can start as soon as xs0 lands,
            # then the bulk.
            if mode != "pe":
                for h, (k0, kn) in enumerate(W_CHUNKS):
                    wk = weffp.tile([P, kn, O], BF16, tag=f"wc{h}", name=f"wc{h}")
                    w_dmas.append(nc.scalar.dma_start(wk[:], wf_r[:, k0 : k0 + kn, :]))
                    for j in range(kn):
                        weff.append(wk[:, j, :])

            # Effective bias, replicated on every partition: the DMA reads the
            # (1, O) bias row once per partition via a 0-stride AP. Rides the
            # ACT ring behind the weight chunks (not needed until first drain).
            if mode in ("full", "dma"):
                bb_t = consts.tile([P, 1, O], F32)
                nc.scalar.dma_start(bb_t[:], bf.ap().partition_broadcast(P))
                bias_t = bb_t[:, 0, :]

            # Main loop: stream token columns, matmul against the resident
            # W_eff in bf16 (full-rate path, fp32 PSUM), bias + sigmoid, store.
            #
            # ss=0 runs its 8 PSUM accumulation groups k-outer (wave per
            # contraction tile) so the PE consumes each weff[k] the moment it
            # lands instead of serializing whole groups behind weff[7].
            out_r = out.ap().rearrange("(c p) o -> p c o", p=P)
            groups = [(sub, oh) for sub in range(S_SUB) for oh in range(OH)]
            for ss in range(N_SUPER):
                if ss == 0 or mode == "pe":
                    xs = xs0
                else:
                    cols = slice(ss * S_SUPER, (ss + 1) * S_SUPER)
                    xs = xpool.tile([P, KT, S_SUPER], BF16, tag="xs")
                    d = nc.sync.dma_start(xs[:], xT_r[:, :, cols])
                    if X_YIELDS_TO_W and ss == 1 and w_dmas:
                        # Keep the early prefetch from stealing HBM
                        # bandwidth while the weight chunks stream in.
                        tile.add_dep_helper(
                            d.ins, w_dmas[-1].ins, sync=True,
                            reason="x prefetch yields to weight prefix",
                        )
                if mode == "pe":
                    ostage = None
                elif mode == "dma":
                    ostage = ost_cal
                else:
                    ostage = opool.tile([P, S_SUB, O], OUT_DT)

                def drain(ps, sub, oh):
                    if mode != "full":
                        return
                    osl = slice(oh * ON, (oh + 1) * ON)
                    nc.vector.tensor_add(ps[:], ps[:], bias_t[:, osl])
                    nc.scalar.activation(
                        ostage[:, sub, osl], ps[:],
                        mybir.ActivationFunctionType.Sigmoid,
                    )

                def store():
                    if not DO_DMA:
                        return
                    if ss == N_SUPER - 1:
                        for sub in range(S_SUB):
                            nc.scalar.dma_start(
                                out_r[:, ss * S_SUB + sub, :], ostage[:, sub, :]
                            )
                    else:
                        nc.scalar.dma_start(
                            out_r[:, ss * S_SUB : (ss + 1) * S_SUB, :], ostage[:]
                        )

                if not DO_PE:
                    for g, (sub, oh) in enumerate(groups):
                        drain(None, sub, oh)
                    store()
                elif ss == 0:
                    pss = [pspool.tile([P, ON], F32, name=f"ps{g}", tag=f"ps{g}") for g in range(len(groups))]
                    for k in range(KT):
                        for g, (sub, oh) in enumerate(groups):
                            nc.tensor.matmul(
                                pss[g],
                                xs[:, k, sub * P : (sub + 1) * P],
                                weff[k][:, oh * ON : (oh + 1) * ON],
                                start=(k == 0),
                                stop=(k == KT - 1),
                            )
                    for g, (sub, oh) in enumerate(groups):
                        drain(pss[g], sub, oh)
                    store()
                else:
                    for g, (sub, oh) in enumerate(groups):
                        ps = pspool.tile([P, ON], F32, name=f"ps{g}", tag=f"ps{g}")
                        for k in range(KT):
                            nc.tensor.matmul(
                                ps[:],
                                xs[:, k, sub * P : (sub + 1) * P],
                                weff[k][:, oh * ON : (oh + 1) * ON],
                                start=(k == 0),
                                stop=(k == KT - 1),
                            )
                        drain(ps, sub, oh)
                    store()

        if tok is not None:
            nc.sync.dma_start(tok.ap(), pb.ap()[0:1, :])

    nc.compile()
    return nc


def _get_program():
    global _PROGRAM
    if _PROGRAM is None:
        _PROGRAM = _build_program()
    return _PROGRAM


def _make_in_maps(tensor, sel_idx, sel_probs, W, b):
    tensor = np.asarray(tensor, dtype=np.float32)
    sel_idx = np.asarray(sel_idx).astype(np.int64)
    sel_probs = np.asarray(sel_probs, dtype=np.float32)
    W = np.asarray(W, dtype=np.float32)
    b = np.asarray(b, dtype=np.float32)

    in_maps = []
    for c in range(N_CORES):
        idx = sel_idx[c]
        p = sel_probs[c]
        # Bank gather + superposition on host (2 x 4 MB per batch, trivial):
        # the device sees only the 2 MB effective weight matrix.
        weff = p[0] * W[idx[0]] + p[1] * W[idx[1]]          # (D, O) fp32
        beff = p[0] * b[idx[0]] + p[1] * b[idx[1]]          # (O,)   fp32
        in_maps.append(
            {
                "xT": np.ascontiguousarray(tensor[c].T).astype(BF16_NP),
                "wf": weff.astype(BF16_NP),
                "pb": np.ascontiguousarray(
                    np.broadcast_to(p[None, :], (P, TOP_K))
                ),
                "bf": beff[None, :],
            }
        )
    return in_maps


def _execute(in_maps, trace=False, **kwargs):
    nc = _get_program()
    return run_bass_kernel_spmd(
        nc, in_maps, core_ids=list(range(N_CORES)), trace=trace, **kwargs
    )


def kernel(tensor, sel_idx, sel_probs, W, b):
    in_maps = _make_in_maps(tensor, sel_idx, sel_probs, W, b)
    res = _execute(in_maps)
    out = np.stack(
        [res.results[c]["out"].astype(np.float32) for c in range(N_CORES)], axis=0
    )
    return out.reshape(B, S, NUM_HEADS, D_HEAD)
